# revision 53
# baseline (speedup 1.0000x reference)
"""MultiHeadAttention Trainium2 kernel (v4).

Sharding: 8 cores = 4 batches x 2 head-groups (8 heads each).
Each core computes, for its (batch b, head group gi):
  Q = q[b] @ Wq[:, gi*512:+512] + bq_g        (same fc applied to k, v)
  per head: softmax(QK^T/8 with mask) @ V
  partial_out = attn @ Wo[gi*512:+512, :]
Host sums the two partial outputs per batch and adds b_o.

v4 (structural change vs v3): the attn@V accumulation is FLIPPED so the
cost-model-charged free dimension is the head dim (65 incl. denominator
column) instead of sq (512):
  - acc[sq 128, s(4 sq-subtiles), 65] per (pr, head) accumulates
    probs^T-slices (lhsT [sk,128]) @ [V_head | ones] over the 16 sk tiles.
    One PSUM accumulation group per 2KB bank (start only on the first
    write, other slots zero-on-first-touch inside the zero region).
  - softmax denominator lands per sq-row (the 65th column) -> per-
    partition normalization on DVE (reciprocal + stride-0 broadcast
    mul), no partition_broadcast needed.
  - attn returns to [dh, sq] layout for the output projection via the
    XBAR DMA transpose (SBUF->SBUF, 4x [128,128] tiles per pr) on the
    sync queue; no PE transposes, no extra PSUM.
  This halves attnV PE time (the 16-step contraction now multiplies a
  65-wide output instead of 512-wide): chunks 1-3 then run exactly at
  the ACT exp pace (1038ns/beat) and chunk 0 is PE-bound at ~95% while
  it absorbs the K/V/Q projections.

Other v4 changes:
  - mask DMAs split across the gpsimd(SWDGE) and sync(HWDGE) queues
    (cross-queue DMA transfers overlap; same-queue serializes end-to-
    end, and HWDGE DMAs hold the issuing SEQ during their sem waits so
    the scalar/ACT queue must never carry stream-time traffic).
  - xq is loaded as per-chunk [128,512] column slices (own ring) so xk
    and xv stay resident permanently: no re-load, and K d-tiles 1-3 are
    projected from the original xk inside chunk-0 slot beats with
    deadlines matched to the score stream (kt d-tile dt is read by
    scores of pr=dt).
  - lead-in only computes K d-tile 0, V sk-tiles 0-6 and Q(c0) dt0,
    db-interleaved so the PE consumes x tiles as they land; a junk-
    matmul warmup bridges the DMA wait so pe_busy_start predates the
    real work by >3us (full 2.4GHz from the first piece).  Everything
    else drains into chunk-0 slot/window beats; the exp stream starts
    ~26us in.
  - K-projection bias/cast moved from ACT to DVE (ACT runs only the exp
    stream).
  - PSUM: mm ring 2x[128,2,512] f32 (score pairs + pieces) and acc
    ring 4x 2KB banks (attnV accumulators + [128,512] window pieces at
    t in {5,6,11,12}: ring order [h0, h1, w5, w11] so a window alloc
    never evicts a live accumulator).
  - tail: O(last chunk) runs pr0-2 partials during the final beats,
    closes with pr3 after the last norm whose XBAR transposes go to the
    then-idle scalar queue; kk2/kk3 use 1024-wide 2-bank psums so both
    column halves accumulate concurrently; stores alternate sync/gpsimd.

Device layout (per core):
  - inputs arrive TRANSPOSED: xT [1024, seq] bf16
  - Q^T, K^T stored [128, 4, *] bf16 (partition = d within d-tile; head g
    occupies partitions 64*(g%2).. of d-tile g//2)
  - scores computed transposed S^T[sk, sq], two heads packed in the PE
    array via tile_position row tiling (K=64 each)
  - softmax without max-subtraction (scores bounded ~|6| after 1/8 scale)
  - mask applied multiplicatively AFTER exp (notmask in {0,1} bf16)
"""

import sys

import numpy as np
import ml_dtypes

try:
    import concourse.bass as bass  # noqa: F401
except ImportError:  # pragma: no cover
    for _p in ("/opt/trn_rl_repo", "/root/.axon_site/_ro/trn_rl_repo"):
        if _p not in sys.path:
            sys.path.insert(0, _p)
    import concourse.bass as bass  # noqa: F401

import concourse.tile as tile
from concourse import bacc, mybir
from concourse.bass_utils import run_bass_kernel_spmd

BF16 = ml_dtypes.bfloat16

D_MODEL = 1024
N_HEADS = 16
BATCH = 4
SEQ = 2048
DH = 64           # head dim
HG = 8            # heads per core
DG = HG * DH      # 512, projected dim per core

F32 = mybir.dt.float32
BF16D = mybir.dt.bfloat16

TG = 2            # sk tiles per mask DMA group
LAG = 4           # attnV beats behind exp
WIN_T = (5, 6, 11, 12)   # acc-ring piece window beats (allocs at 5 and 11)


def build_nc(seq=SEQ):
    """Build the per-core SPMD Bass program."""
    assert seq % 512 == 0
    NT = seq // 128       # sk tiles (16)
    NC_ = seq // 512      # sq chunks (4)
    NG = NT // TG         # mask groups per (c, pr) (8)
    NB = NC_ * 4 * NT     # total beats (256)
    BEATS = [(c, pr, t) for c in range(NC_) for pr in range(4)
             for t in range(NT)]

    nc = bacc.Bacc(None, target_bir_lowering=False)

    xqT = nc.dram_tensor("xqT", [D_MODEL, seq], BF16D, kind="ExternalInput")
    xkT = nc.dram_tensor("xkT", [D_MODEL, seq], BF16D, kind="ExternalInput")
    xvT = nc.dram_tensor("xvT", [D_MODEL, seq], BF16D, kind="ExternalInput")
    wq = nc.dram_tensor("wq", [D_MODEL, DG], BF16D, kind="ExternalInput")
    bq = nc.dram_tensor("bq", [DG], F32, kind="ExternalInput")
    wo = nc.dram_tensor("wo", [DG, D_MODEL], BF16D, kind="ExternalInput")
    # notmask, transposed + tiled: [c, pr, g, p(sk in tile), t(in group), h, j]
    nm = nc.dram_tensor("nm", [NC_, 4, NG, 128, TG, 2, 512], BF16D,
                        kind="ExternalInput")
    out = nc.dram_tensor("out", [seq, D_MODEL], F32, kind="ExternalOutput")

    EXP = mybir.ActivationFunctionType.Exp

    with tile.TileContext(nc) as tc:
        with tc.tile_pool(name="persist", bufs=1) as persist, \
             tc.tile_pool(name="qtp", bufs=2) as qtp, \
             tc.tile_pool(name="atp", bufs=2) as atp, \
             tc.tile_pool(name="xpool", bufs=16) as xpool, \
             tc.tile_pool(name="nmp", bufs=9) as nmp, \
             tc.tile_pool(name="probsp", bufs=8) as probsp, \
             tc.tile_pool(name="tmpp", bufs=2) as tmpp, \
             tc.tile_pool(name="rp", bufs=4) as rp, \
             tc.tile_pool(name="osbp", bufs=3) as osbp, \
             tc.tile_pool(name="mmps", bufs=2, space="PSUM") as mmps, \
             tc.tile_pool(name="accp", bufs=4, space="PSUM") as accp:

            kt_sb = persist.tile([128, 4, seq], BF16D, name="kt_sb")
            v_sb = persist.tile([128, NT, HG, DH + 1], BF16D, name="v_sb")
            wo_sb = persist.tile([128, 4, D_MODEL], BF16D, name="wo_sb")
            wq_sb = persist.tile([128, 8, DG], BF16D, name="wq_sb")
            bq_sb = persist.tile([128, 4], F32, name="bq_sb")
            bqrep = persist.tile([128, HG, DH], F32, name="bqrep")

            nc.sync.dma_start(out=bq_sb, in_=bq.rearrange("(t p) -> p t", p=128))
            nc.vector.memset(v_sb[:, :, :, DH:DH + 1], 1.0)

            # PE p-state warmup: junk matmuls bridge the xk DMA wait so
            # pe_busy_start predates the real pieces by >3us and they run at
            # the full 2.4GHz clock (the sim resets the ramp after long PE
            # idle gaps, so the chain must reach the first kpiece)
            warm = persist.tile([128, 512], BF16D, name="warm")
            nc.vector.memset(warm, 0.5)
            wps = accp.tile([64, 512], F32, name="wps", tag="acc")
            for i in range(6):
                nc.tensor.matmul(wps, warm[:, 0:64], warm[:, :],
                                 start=(i == 0), stop=(i == 5))

            def load_x(xh, pref, queues=None):
                ts_ = []
                for db in range(8):
                    xt = xpool.tile([128, seq], BF16D, name=f"{pref}{db}", tag="x")
                    if queues is None:
                        q = nc.sync if db % 2 == 0 else nc.scalar
                    else:
                        q = queues[db % len(queues)]
                    q.dma_start(out=xt, in_=xh[db * 128:(db + 1) * 128, :])
                    ts_.append(xt)
                return ts_

            xqs_d = {}

            def load_xqs(c, queues=(None, None)):
                """Chunk-column slices of xq ([128,512] per db-tile): Q reads
                only its chunk's columns, so xq never needs full residency."""
                ts_ = []
                for db in range(8):
                    xt = xpool.tile([128, 512], BF16D, name=f"xq{c}_{db}",
                                    tag="xqs")
                    q = queues[db % 2] or (nc.sync if db % 2 == 0 else nc.scalar)
                    q.dma_start(out=xt,
                                in_=xqT[db * 128:(db + 1) * 128,
                                        c * 512:(c + 1) * 512])
                    ts_.append(xt)
                xqs_d[c] = ts_

            # projection piece emitters ------------------------------------
            def kpiece(dt, half, xts, width=512, tag="mm"):
                """K^T projection for d-tile dt, `width` seq cols from
                half*width (one [128,width] psum)."""
                h0 = half * width
                pool = mmps if tag == "mm" else accp
                ps = pool.tile([128, width], F32, name="kps", tag=tag)
                for db in range(8):
                    nc.tensor.matmul(
                        ps,
                        wq_sb[:, db, dt * 128:(dt + 1) * 128],
                        xts[db][:, h0:h0 + width],
                        start=(db == 0), stop=(db == 7),
                    )
                nc.vector.tensor_scalar_add(
                    kt_sb[:, dt, h0:h0 + width], ps, bq_sb[:, dt:dt + 1],
                )

            def kpair(dt, e0, xts):
                """Two 256-wide K^T halves sharing one [128,512] acc-ring
                psum sequentially (window piece, 2 parts)."""
                state = {}

                def part(i):
                    if "ps" not in state:
                        state["ps"] = accp.tile([128, 512], F32,
                                                name="kpw", tag="acc")
                    ps = state["ps"][:, 0:256]
                    h0 = (e0 + i) * 256
                    for db in range(8):
                        nc.tensor.matmul(
                            ps,
                            wq_sb[:, db, dt * 128:(dt + 1) * 128],
                            xts[db][:, h0:h0 + 256],
                            start=(db == 0), stop=(db == 7),
                        )
                    nc.vector.tensor_scalar_add(
                        kt_sb[:, dt, h0:h0 + 256], ps, bq_sb[:, dt:dt + 1],
                    )
                return [lambda i=i: part(i) for i in range(2)]

            qt_d = {}

            def _qt(c):
                if c not in qt_d:
                    qt_d[c] = qtp.tile([128, 4, 512], BF16D,
                                       name=f"qt{c}", tag="qt")
                return qt_d[c]

            def qpiece(c, dt, tag="mm"):
                """Q projection for chunk c, one d-tile (one [128,512] psum),
                reading the chunk-sliced xq tiles."""
                qt_c = _qt(c)
                pool = mmps if tag == "mm" else accp
                ps = pool.tile([128, 512], F32, name="qps", tag=tag)
                for db in range(8):
                    nc.tensor.matmul(
                        ps,
                        wq_sb[:, db, dt * 128:(dt + 1) * 128],
                        xqs_d[c][db][:, :],
                        start=(db == 0), stop=(db == 7),
                    )
                nc.vector.tensor_scalar_add(
                    qt_c[:, dt, :], ps, bq_sb[:, dt:dt + 1],
                )

            def qpiece_parts(c, dt):
                """Q projection piece for (c, dt): 2 part-closures (~853ns PE
                each) sharing one [128,512] acc-ring psum."""
                state = {}

                def part(dbh):
                    if "ps" not in state:
                        state["qt"] = _qt(c)
                        state["ps"] = accp.tile([128, 512], F32,
                                                name="qpsw", tag="acc")
                    ps = state["ps"]
                    for db in range(4 * dbh, 4 * dbh + 4):
                        nc.tensor.matmul(
                            ps,
                            wq_sb[:, db, dt * 128:(dt + 1) * 128],
                            xqs_d[c][db][:, :],
                            start=(db == 0), stop=(db == 7),
                        )
                    if dbh == 1:
                        nc.vector.tensor_scalar_add(
                            state["qt"][:, dt, :], ps, bq_sb[:, dt:dt + 1],
                        )
                return [lambda dbh=dbh: part(dbh) for dbh in range(2)]

            def vpiece(st, xts, tag="mm"):
                """V projection for one sk tile (one [128,512] psum)."""
                pool = mmps if tag == "mm" else accp
                ps = pool.tile([128, DG], F32, name="vps", tag=tag)
                for db in range(8):
                    nc.tensor.matmul(
                        ps, xts[db][:, st * 128:(st + 1) * 128], wq_sb[:, db, :],
                        start=(db == 0), stop=(db == 7),
                    )
                nc.vector.tensor_add(
                    v_sb[:, st, :, 0:DH],
                    ps[:, :].rearrange("p (g e) -> p g e", g=HG),
                    bqrep,
                )

            at_d = {}

            def _at(c):
                if c not in at_d:
                    at_d[c] = atp.tile([128, 4, 512], BF16D,
                                       name=f"at{c}", tag="at")
                return at_d[c]

            def opiece_pair(c, kk, tag="acc"):
                """Output projection rows [c*512+kk*128, +128): both 512-col
                halves as 2 part-closures SHARING one [128,512] psum (the
                second group's matmuls WAR-wait on the first's copy-out)."""
                row = c * 512 + kk * 128
                state = {}

                def part(dch):
                    if "ps" not in state:
                        pool = accp if tag == "acc" else mmps
                        state["ps"] = pool.tile([128, 512], F32,
                                                name="opsw", tag=tag)
                    ops = state["ps"]
                    for pr in range(4):
                        nc.tensor.matmul(
                            ops,
                            at_d[c][:, pr, kk * 128:(kk + 1) * 128],
                            wo_sb[:, pr, dch * 512:(dch + 1) * 512],
                            start=(pr == 0), stop=(pr == 3),
                        )
                    osb = osbp.tile([128, 512], F32, name="osb", tag="osb")
                    nc.vector.tensor_copy(osb, ops)
                    nc.sync.dma_start(
                        out=out[row:row + 128, dch * 512:(dch + 1) * 512],
                        in_=osb)
                return [lambda dch=dch: part(dch) for dch in range(2)]

            # ---- lead-in --------------------------------------------------
            # only what pr0 of the stream needs up front: K d-tile 0, V
            # sk-tiles 0-8 and Q(c0) d-tile 0.  Everything else (V 9-15,
            # Q(c0) d-tiles 1-3, K d-tiles 1-3) drains into chunk-0
            # slot/window beats so the exp stream starts ~26us in.
            nmt_d = {p: [None] * NG for p in range(16)}
            nmq = [0]

            def emit_nmt_group(p, g, q=None):
                # gpsimd/sync split: HWDGE DMAs hold the issuing SEQ during
                # their sem waits, so the scalar (ACT) queue must stay clear
                # of mask traffic or the exp stream stalls ~1.6us per group
                c, pr = divmod(p, 4)
                nmt = nmp.tile([128, TG, 2, 512], BF16D, name="nmt", tag="nmt")
                if q is None:
                    q = nc.gpsimd if nmq[0] % 2 == 0 else nc.sync
                    nmq[0] += 1
                q.dma_start(out=nmt, in_=nm[c, pr, g])
                nmt_d[p][g] = nmt

            # wq halves first (they gate every projection), then xk and xv
            # striped across all three queues; pr0 masks go to gpsimd after
            # the x loads — they have ~20us of slack
            _wqr = wq.rearrange("(n p) m -> p n m", p=128)
            nc.sync.dma_start(out=wq_sb[:, 0:4, :], in_=_wqr[:, 0:4, :])
            nc.scalar.dma_start(out=wq_sb[:, 4:8, :], in_=_wqr[:, 4:8, :])
            xk_ts = load_x(xkT, "xk", queues=(nc.gpsimd, nc.scalar, nc.sync))
            xv_ts = load_x(xvT, "xv", queues=(nc.gpsimd, nc.sync, nc.scalar))
            load_xqs(0)
            for g in range(NG):
                emit_nmt_group(0, g, q=nc.gpsimd)
            _bqap = bq[:].rearrange("(g e) -> g e", g=HG)
            nc.scalar.dma_start(out=bqrep, in_=bass.AP(
                tensor=_bqap.tensor, offset=_bqap.offset,
                ap=[[0, 128]] + [list(d) for d in _bqap.ap]))
            load_xqs(1)
            nc.scalar.dma_start(out=wo_sb, in_=wo.rearrange("(n p) m -> p n m", p=128))

            # K d-tile 0 with the db contraction outermost: each db-round
            # consumes its xk tile as it lands (4 quarter-psums in flight)
            kps0 = [mmps.tile([128, 512], F32, name="kps", tag="mm"),
                    accp.tile([128, 512], F32, name="kps", tag="acc"),
                    accp.tile([128, 512], F32, name="kps", tag="acc"),
                    mmps.tile([128, 512], F32, name="kps", tag="mm")]
            for db in range(8):
                for quarter in range(4):
                    nc.tensor.matmul(
                        kps0[quarter],
                        wq_sb[:, db, 0:128],
                        xk_ts[db][:, quarter * 512:(quarter + 1) * 512],
                        start=(db == 0), stop=(db == 7),
                    )
            for quarter in range(4):
                nc.vector.tensor_scalar_add(
                    kt_sb[:, 0, quarter * 512:(quarter + 1) * 512],
                    kps0[quarter], bq_sb[:, 0:1],
                )
            # V 0-6 in db-interleaved pairs; Q(c0) dt0 rides the V2 round so
            # its bias-add latency hides under the V4-V6 matmuls
            qt0 = _qt(0)
            q0ps = None
            vps = {}
            for st0 in (0, 2, 4):
                vps[st0] = [mmps.tile([128, DG], F32, name="vps", tag="mm"),
                            accp.tile([128, DG], F32, name="vps", tag="acc")]
                if st0 == 2:
                    q0ps = accp.tile([128, 512], F32, name="qps", tag="acc")
                for db in range(8):
                    for i in range(2):
                        nc.tensor.matmul(
                            vps[st0][i],
                            xv_ts[db][:, (st0 + i) * 128:(st0 + i + 1) * 128],
                            wq_sb[:, db, :],
                            start=(db == 0), stop=(db == 7),
                        )
                    if st0 == 2:
                        nc.tensor.matmul(
                            q0ps, wq_sb[:, db, 0:128], xqs_d[0][db][:, :],
                            start=(db == 0), stop=(db == 7),
                        )
                for i in range(2):
                    nc.vector.tensor_add(
                        v_sb[:, st0 + i, :, 0:DH],
                        vps[st0][i][:, :].rearrange("p (g e) -> p g e", g=HG),
                        bqrep,
                    )
                if st0 == 2:
                    nc.vector.tensor_scalar_add(qt0[:, 0, :], q0ps,
                                                bq_sb[:, 0:1])
            v6ps = mmps.tile([128, DG], F32, name="vps", tag="mm")
            for db in range(8):
                nc.tensor.matmul(
                    v6ps, xv_ts[db][:, 6 * 128:7 * 128], wq_sb[:, db, :],
                    start=(db == 0), stop=(db == 7),
                )
            nc.vector.tensor_add(
                v_sb[:, 6, :, 0:DH],
                v6ps[:, :].rearrange("p (g e) -> p g e", g=HG), bqrep)

            # ---- flattened beat stream -----------------------------------
            spair_d, probs_d, acc_d = {}, {}, {}

            def emit_s(b):
                c, pr, t = BEATS[b]
                sp = mmps.tile([128, 2, 512], F32, name="spair", tag="mm")
                spair_d[b] = sp
                tc_cols = slice(t * 128, (t + 1) * 128)
                nc.tensor.matmul(
                    sp[:, 0, :], kt_sb[0:64, pr, tc_cols],
                    qt_d[c][0:64, pr, :], start=True, stop=True,
                    tile_position=(0, 0),
                )
                nc.tensor.matmul(
                    sp[:, 1, :], kt_sb[64:128, pr, tc_cols],
                    qt_d[c][64:128, pr, :], start=True, stop=True,
                    tile_position=(64, 0),
                )

            def emit_exp_mask(b):
                c, pr, t = BEATS[b]
                p = 4 * c + pr
                probs = probsp.tile([128, 2, 512], BF16D, name="probs",
                                    tag="probs")
                probs_d[b] = probs
                nc.scalar.activation(probs, spair_d[b], EXP, scale=0.125)
                nc.vector.tensor_mul(probs, probs,
                                     nmt_d[p][t // TG][:, t % TG, :, :])

            def emit_av(b):
                c, pr, t = BEATS[b]
                p = 4 * c + pr
                if t == 0:
                    acc_d[p] = [accp.tile([128, 4, 128], F32, name="acc",
                                          tag="acc") for _ in range(2)]
                for h in range(2):
                    acc = acc_d[p][h]
                    for s in range(4):
                        nc.tensor.matmul(
                            acc[:, s, 0:DH + 1],
                            probs_d[b][:, h, s * 128:(s + 1) * 128],
                            v_sb[:, t, 2 * pr + h, :],
                            start=(t == 0 and s == 0),
                            stop=(t == NT - 1 and s == 3),
                        )

            def emit_norm(c, pr):
                p = 4 * c + pr
                at_c = _at(c)
                tmpt = tmpp.tile([128, 4, 2, DH], BF16D, name="tmpt",
                                 tag="tmpt")
                for h in range(2):
                    acc = acc_d[p][h]
                    rcp = rp.tile([128, 4], F32, name="rcp", tag="rcp")
                    nc.vector.reciprocal(rcp, acc[:, :, DH])
                    rap = rcp[:, :]
                    bc = bass.AP(tensor=rap.tensor, offset=rap.offset,
                                 ap=[list(rap.ap[0]), list(rap.ap[1]),
                                     [0, DH]])
                    nc.vector.tensor_mul(tmpt[:, :, h, :], acc[:, :, 0:DH], bc)
                # the last pr's transposes go to the scalar queue (idle once
                # the exp stream has ended; the sync queue is still draining
                # output stores, which would delay at(pr3) and the tail)
                q = nc.scalar if p == 15 else nc.sync
                for s in range(4):
                    q.dma_start(
                        out=at_c[:, pr, s * 128:(s + 1) * 128],
                        in_=tmpt[:, s, :, :], transpose=True)

            # chunk-0 slot pieces, ordered so every piece's EMISSION beat
            # precedes the emission of the first score matmul reading its
            # output (kt d-tile dt is read by scores of pr=dt, so K d-tiles
            # have progressive deadlines through chunk 0):
            #   pr0 t1..15 odd: Q(c0)dt1, V9..V15
            #   pr1 t1,3:      K dt1 cols 1536:2048, 1024:1536 (512-wide)
            #   pr1 t5..pr2 t3: K dt2 (256-wide x8)
            #   pr2 t5..pr3 t3: K dt3 (256-wide x8)
            small, win = [], []
            small.append(lambda: qpiece(0, 1))
            for st in range(7, NT):
                small.append(lambda st=st: vpiece(st, xv_ts, tag="mm"))
            small.append(lambda: kpiece(1, 2, xk_ts, width=512))
            small.append(lambda: kpiece(1, 3, xk_ts, width=512))
            for dt in (2, 3):
                for e in range(8):
                    small.append(lambda dt=dt, e=e:
                                 kpiece(dt, e, xk_ts, width=256))

            def drain1():
                if small:
                    small.pop(0)()

            next_s = 0
            for b in range(NB + LAG):
                if b < NB:
                    c, pr, t = BEATS[b]
                    p = 4 * c + pr
                    if t == 0 and pr == 0:
                        # queue this chunk's window pieces: 2-part pieces in
                        # acc-ring banks, allocated only at t in {5, 11}
                        # (ring order [h0, h1, w5, w11] so a window alloc
                        # never lands on a live accumulator)
                        if c == 0:
                            win.extend(kpair(1, 0, xk_ts))
                            win.extend(kpair(1, 2, xk_ts))
                            win.extend(qpiece_parts(0, 2))
                            win.extend(qpiece_parts(0, 3))
                            for dt in range(4):
                                win.extend(qpiece_parts(1, dt))
                        else:
                            if c + 1 < NC_:
                                for dt in range(4):
                                    win.extend(qpiece_parts(c + 1, dt))
                            for kk in range(4):
                                win.extend(opiece_pair(c - 1, kk))
                    # xq chunk-slice prefetch (two chunks ahead; keep these
                    # off the scalar queue so ACT SEQ never blocks)
                    if pr == 2 and t == 0 and c + 2 < NC_:
                        load_xqs(c + 2, queues=(nc.sync, nc.gpsimd))
                    # mask prefetch, one pr ahead, spread over even beats
                    if p + 1 < 16:
                        if t == 0:
                            emit_nmt_group(p + 1, 0)
                            emit_nmt_group(p + 1, 1)
                        elif t % 2 == 0 and t <= 12:
                            emit_nmt_group(p + 1, t // 2 + 1)
                    # mm drain slots (chunk 0 is PE-bound; ACT runs ahead)
                    slot_ok = t % 2 == 1 if c == 0 else t == 7
                    is_slot = slot_ok and small
                    # scores run 2 beats ahead of the exp stream (only to b
                    # on slot beats: the piece borrows the spair ring slot)
                    cap = b if is_slot else b + 2
                    while next_s <= cap and next_s < NB:
                        emit_s(next_s)
                        next_s += 1
                    emit_exp_mask(b)
                    if is_slot:
                        # one piece psum reuses a spair ring slot whose exp
                        # is in flight right now; scores catch up next beat
                        drain1()
                    if t in WIN_T and win:
                        win.pop(0)()
                if b >= LAG:
                    bb = b - LAG
                    emit_av(bb)
                    cb, prb, tb = BEATS[bb]
                    if tb == NT - 1:
                        emit_norm(cb, prb)
            while win:
                win.pop(0)()
            while small:
                small.pop(0)()
            # tail: output projection of the last chunk, 4 psum groups deep;
            # the pr3 contractions are emitted last so pr0-2 partials cover
            # the final norm/transpose latency.  ACT and Pool are idle once
            # the stream ends, so copies alternate DVE/ACT and stores
            # alternate sync/gpsimd to halve the drain.
            cl = NC_ - 1
            o_ps = [accp.tile([128, 512], F32, name="otp", tag="acc"),
                    accp.tile([128, 512], F32, name="otp", tag="acc"),
                    mmps.tile([128, 2, 512], F32, name="otp", tag="mm"),
                    mmps.tile([128, 2, 512], F32, name="otp", tag="mm")]

            def _ops(kk, dch):
                return o_ps[kk] if kk < 2 else o_ps[kk][:, dch, :]

            def _omm(kk, dch, pr, start, stop):
                nc.tensor.matmul(
                    _ops(kk, dch),
                    at_d[cl][:, pr, kk * 128:(kk + 1) * 128],
                    wo_sb[:, pr, dch * 512:(dch + 1) * 512],
                    start=start, stop=stop)

            def _oclose(kk, dch):
                _omm(kk, dch, 3, False, True)
                row = cl * 512 + kk * 128
                osb = osbp.tile([128, 512], F32, name="osbt", tag="osb")
                nc.vector.tensor_copy(osb, _ops(kk, dch))
                q = nc.sync if (kk + dch) % 2 == 0 else nc.gpsimd
                q.dma_start(
                    out=out[row:row + 128, dch * 512:(dch + 1) * 512],
                    in_=osb)

            # pr0-2 partials first (kk2/kk3 carry both column halves in
            # 2-bank psums so they never serialize on a copy-out), then the
            # pr3 closers as at(pr3) lands, then kk0/kk1's second halves
            for pr in range(3):
                for kk in range(4):
                    _omm(kk, 0, pr, pr == 0, False)
            for pr in range(3):
                for kk in (2, 3):
                    _omm(kk, 1, pr, pr == 0, False)
            for kk in range(4):
                _oclose(kk, 0)
            for kk in (2, 3):
                _oclose(kk, 1)
            for kk in (0, 1):
                for pr in range(3):
                    _omm(kk, 1, pr, pr == 0, False)
            for kk in (0, 1):
                _oclose(kk, 1)

    nc.compile()
    return nc


_NC_CACHE = {}


def _get_nc(seq=SEQ):
    if seq not in _NC_CACHE:
        _NC_CACHE[seq] = build_nc(seq)
    return _NC_CACHE[seq]


def make_core_inputs(q, k, v, mask, W_q, b_q, W_o, seq=SEQ):
    """Build the 8 per-core input maps (host-side shard + layout)."""
    NT = seq // 128
    NC_ = seq // 512
    NG = NT // TG
    in_maps = []
    notm_all = (~np.asarray(mask)).astype(BF16)  # [B, 16, sq, sk]
    for core in range(8):
        b, gi = divmod(core, 2)
        cols = slice(gi * DG, (gi + 1) * DG)
        xqT = np.ascontiguousarray(np.asarray(q[b], np.float32).T.astype(BF16))
        xkT = np.ascontiguousarray(np.asarray(k[b], np.float32).T.astype(BF16))
        xvT = np.ascontiguousarray(np.asarray(v[b], np.float32).T.astype(BF16))
        wqc = np.ascontiguousarray(np.asarray(W_q, np.float32)[:, cols]).astype(BF16)
        bqc = np.ascontiguousarray(np.asarray(b_q, np.float32)[cols])
        woc = np.ascontiguousarray(np.asarray(W_o, np.float32)[cols, :]).astype(BF16)
        nmc = notm_all[b, gi * HG:(gi + 1) * HG]  # [8, sq, sk] bf16
        # [8h, sq, sk] -> [c, pr, g, p, t, h, j]
        # h -> (pr 4, h2); sq -> (c 4, j 512); sk -> (g NG, t TG, p 128)
        nmc = np.ascontiguousarray(
            nmc.reshape(4, 2, NC_, 512, NG, TG, 128)
               .transpose(2, 0, 4, 6, 5, 1, 3)
        )
        in_maps.append({
            "xqT": xqT, "xkT": xkT, "xvT": xvT,
            "wq": wqc, "bq": bqc, "wo": woc, "nm": nmc,
        })
    return in_maps


def kernel(q, k, v, mask, W_q, b_q, W_o, b_o):
    nc = _get_nc(SEQ)
    in_maps = make_core_inputs(q, k, v, mask, W_q, b_q, W_o, SEQ)
    res = run_bass_kernel_spmd(nc, in_maps, core_ids=list(range(8)))
    out = np.empty((BATCH, SEQ, D_MODEL), np.float32)
    bo = np.asarray(b_o, np.float32)
    for b in range(BATCH):
        out[b] = res.results[2 * b]["out"] + res.results[2 * b + 1]["out"] + bo
    return out


# revision 63
# speedup vs baseline: 1.0023x; 1.0023x over previous
"""MultiHeadAttention Trainium2 kernel (v4).

Sharding: 8 cores = 4 batches x 2 head-groups (8 heads each).
Each core computes, for its (batch b, head group gi):
  Q = q[b] @ Wq[:, gi*512:+512] + bq_g        (same fc applied to k, v)
  per head: softmax(QK^T/8 with mask) @ V
  partial_out = attn @ Wo[gi*512:+512, :]
Host sums the two partial outputs per batch and adds b_o.

v4 (structural change vs v3): the attn@V accumulation is FLIPPED so the
cost-model-charged free dimension is the head dim (65 incl. denominator
column) instead of sq (512):
  - acc[sq 128, s(4 sq-subtiles), 65] per (pr, head) accumulates
    probs^T-slices (lhsT [sk,128]) @ [V_head | ones] over the 16 sk tiles.
    One PSUM accumulation group per 2KB bank (start only on the first
    write, other slots zero-on-first-touch inside the zero region).
  - softmax denominator lands per sq-row (the 65th column) -> per-
    partition normalization on DVE (reciprocal + stride-0 broadcast
    mul), no partition_broadcast needed.
  - attn returns to [dh, sq] layout for the output projection via the
    XBAR DMA transpose (SBUF->SBUF, 4x [128,128] tiles per pr) on the
    sync queue; no PE transposes, no extra PSUM.
  This halves attnV PE time (the 16-step contraction now multiplies a
  65-wide output instead of 512-wide): chunks 1-3 then run exactly at
  the ACT exp pace (1038ns/beat) and chunk 0 is PE-bound at ~95% while
  it absorbs the K/V/Q projections.

Other v4 changes:
  - mask DMAs split across the gpsimd(SWDGE) and sync(HWDGE) queues
    (cross-queue DMA transfers overlap; same-queue serializes end-to-
    end, and HWDGE DMAs hold the issuing SEQ during their sem waits so
    the scalar/ACT queue must never carry stream-time traffic).
  - xq is loaded as per-chunk [128,512] column slices (own ring) so xk
    and xv stay resident permanently: no re-load, and K d-tiles 1-3 are
    projected from the original xk inside chunk-0 slot beats with
    deadlines matched to the score stream (kt d-tile dt is read by
    scores of pr=dt).
  - lead-in only computes K d-tile 0, V sk-tiles 0-6 and Q(c0) dt0,
    db-interleaved so the PE consumes x tiles as they land; a junk-
    matmul warmup bridges the DMA wait so pe_busy_start predates the
    real work by >3us (full 2.4GHz from the first piece).  Everything
    else drains into chunk-0 slot/window beats; the exp stream starts
    ~26us in.
  - K-projection bias/cast moved from ACT to DVE (ACT runs only the exp
    stream).
  - PSUM: mm ring 2x[128,2,512] f32 (score pairs + pieces) and acc
    ring 4x 2KB banks (attnV accumulators + [128,512] window pieces at
    t in {5,6,11,12}: ring order [h0, h1, w5, w11] so a window alloc
    never evicts a live accumulator).
  - tail: O(last chunk) runs pr0-2 partials during the final beats,
    closes with pr3 after the last norm whose XBAR transposes go to the
    then-idle scalar queue; kk2/kk3 use 1024-wide 2-bank psums so both
    column halves accumulate concurrently; stores alternate sync/gpsimd.

Device layout (per core):
  - inputs arrive TRANSPOSED: xT [1024, seq] bf16
  - Q^T, K^T stored [128, 4, *] bf16 (partition = d within d-tile; head g
    occupies partitions 64*(g%2).. of d-tile g//2)
  - scores computed transposed S^T[sk, sq], two heads packed in the PE
    array via tile_position row tiling (K=64 each)
  - softmax without max-subtraction (scores bounded ~|6| after 1/8 scale)
  - mask applied multiplicatively AFTER exp (notmask in {0,1} bf16)
"""

import sys

import numpy as np
import ml_dtypes

try:
    import concourse.bass as bass  # noqa: F401
except ImportError:  # pragma: no cover
    for _p in ("/opt/trn_rl_repo", "/root/.axon_site/_ro/trn_rl_repo"):
        if _p not in sys.path:
            sys.path.insert(0, _p)
    import concourse.bass as bass  # noqa: F401

import concourse.tile as tile
from concourse import bacc, mybir
from concourse.bass_utils import run_bass_kernel_spmd

BF16 = ml_dtypes.bfloat16

D_MODEL = 1024
N_HEADS = 16
BATCH = 4
SEQ = 2048
DH = 64           # head dim
HG = 8            # heads per core
DG = HG * DH      # 512, projected dim per core

F32 = mybir.dt.float32
BF16D = mybir.dt.bfloat16

TG = 2            # sk tiles per mask DMA group
LAG = 4           # attnV beats behind exp
WIN_T = (5, 6, 11, 12)   # acc-ring piece window beats (allocs at 5 and 11)


def build_nc(seq=SEQ):
    """Build the per-core SPMD Bass program."""
    assert seq % 512 == 0
    NT = seq // 128       # sk tiles (16)
    NC_ = seq // 512      # sq chunks (4)
    NG = NT // TG         # mask groups per (c, pr) (8)
    NB = NC_ * 4 * NT     # total beats (256)
    BEATS = [(c, pr, t) for c in range(NC_) for pr in range(4)
             for t in range(NT)]

    nc = bacc.Bacc(None, target_bir_lowering=False)

    xqT = nc.dram_tensor("xqT", [D_MODEL, seq], BF16D, kind="ExternalInput")
    xkT = nc.dram_tensor("xkT", [D_MODEL, seq], BF16D, kind="ExternalInput")
    xvT = nc.dram_tensor("xvT", [D_MODEL, seq], BF16D, kind="ExternalInput")
    wq = nc.dram_tensor("wq", [D_MODEL, DG], BF16D, kind="ExternalInput")
    bq = nc.dram_tensor("bq", [DG], F32, kind="ExternalInput")
    wo = nc.dram_tensor("wo", [DG, D_MODEL], BF16D, kind="ExternalInput")
    # notmask, transposed + tiled: [c, pr, g, p(sk in tile), t(in group), h, j]
    nm = nc.dram_tensor("nm", [NC_, 4, NG, 128, TG, 2, 512], BF16D,
                        kind="ExternalInput")
    out = nc.dram_tensor("out", [seq, D_MODEL], F32, kind="ExternalOutput")

    EXP = mybir.ActivationFunctionType.Exp

    with tile.TileContext(nc) as tc:
        with tc.tile_pool(name="persist", bufs=1) as persist, \
             tc.tile_pool(name="qtp", bufs=2) as qtp, \
             tc.tile_pool(name="atp", bufs=2) as atp, \
             tc.tile_pool(name="xpool", bufs=16) as xpool, \
             tc.tile_pool(name="nmp", bufs=9) as nmp, \
             tc.tile_pool(name="probsp", bufs=8) as probsp, \
             tc.tile_pool(name="tmpp", bufs=2) as tmpp, \
             tc.tile_pool(name="rp", bufs=4) as rp, \
             tc.tile_pool(name="osbp", bufs=3) as osbp, \
             tc.tile_pool(name="mmps", bufs=2, space="PSUM") as mmps, \
             tc.tile_pool(name="accp", bufs=4, space="PSUM") as accp:

            kt_sb = persist.tile([128, 4, seq], BF16D, name="kt_sb")
            v_sb = persist.tile([128, NT, HG, DH + 1], BF16D, name="v_sb")
            wo_sb = persist.tile([128, 4, D_MODEL], BF16D, name="wo_sb")
            wq_sb = persist.tile([128, 8, DG], BF16D, name="wq_sb")
            bq_sb = persist.tile([128, 4], F32, name="bq_sb")
            bqrep = persist.tile([128, HG, DH], F32, name="bqrep")

            nc.sync.dma_start(out=bq_sb, in_=bq.rearrange("(t p) -> p t", p=128))
            nc.vector.memset(v_sb[:, :, :, DH:DH + 1], 1.0)

            # PE p-state warmup: junk matmuls bridge the xk DMA wait so
            # pe_busy_start predates the real pieces by >3us and they run at
            # the full 2.4GHz clock (the sim resets the ramp after long PE
            # idle gaps, so the chain must reach the first kpiece)
            warm = persist.tile([128, 512], BF16D, name="warm")
            nc.vector.memset(warm, 0.5)
            wps = accp.tile([64, 512], F32, name="wps", tag="acc")
            for i in range(6):
                nc.tensor.matmul(wps, warm[:, 0:64], warm[:, :],
                                 start=(i == 0), stop=(i == 5))

            def load_x(xh, pref, queues=None):
                ts_ = []
                for db in range(8):
                    xt = xpool.tile([128, seq], BF16D, name=f"{pref}{db}", tag="x")
                    if queues is None:
                        q = nc.sync if db % 2 == 0 else nc.scalar
                    else:
                        q = queues[db % len(queues)]
                    q.dma_start(out=xt, in_=xh[db * 128:(db + 1) * 128, :])
                    ts_.append(xt)
                return ts_

            xqs_d = {}

            def load_xqs(c, queues=(None, None)):
                """Chunk-column slices of xq ([128,512] per db-tile): Q reads
                only its chunk's columns, so xq never needs full residency."""
                ts_ = []
                for db in range(8):
                    xt = xpool.tile([128, 512], BF16D, name=f"xq{c}_{db}",
                                    tag="xqs")
                    q = queues[db % 2] or (nc.sync if db % 2 == 0 else nc.scalar)
                    q.dma_start(out=xt,
                                in_=xqT[db * 128:(db + 1) * 128,
                                        c * 512:(c + 1) * 512])
                    ts_.append(xt)
                xqs_d[c] = ts_

            # projection piece emitters ------------------------------------
            def kpiece(dt, half, xts, width=512, tag="mm"):
                """K^T projection for d-tile dt, `width` seq cols from
                half*width (one [128,width] psum)."""
                h0 = half * width
                pool = mmps if tag == "mm" else accp
                ps = pool.tile([128, width], F32, name="kps", tag=tag)
                for db in range(8):
                    nc.tensor.matmul(
                        ps,
                        wq_sb[:, db, dt * 128:(dt + 1) * 128],
                        xts[db][:, h0:h0 + width],
                        start=(db == 0), stop=(db == 7),
                    )
                nc.vector.tensor_scalar_add(
                    kt_sb[:, dt, h0:h0 + width], ps, bq_sb[:, dt:dt + 1],
                )

            def kpair(dt, e0, xts):
                """Two 256-wide K^T halves sharing one [128,512] acc-ring
                psum sequentially (window piece, 2 parts)."""
                state = {}

                def part(i):
                    if "ps" not in state:
                        state["ps"] = accp.tile([128, 512], F32,
                                                name="kpw", tag="acc")
                    ps = state["ps"][:, 0:256]
                    h0 = (e0 + i) * 256
                    for db in range(8):
                        nc.tensor.matmul(
                            ps,
                            wq_sb[:, db, dt * 128:(dt + 1) * 128],
                            xts[db][:, h0:h0 + 256],
                            start=(db == 0), stop=(db == 7),
                        )
                    nc.vector.tensor_scalar_add(
                        kt_sb[:, dt, h0:h0 + 256], ps, bq_sb[:, dt:dt + 1],
                    )
                return [lambda i=i: part(i) for i in range(2)]

            qt_d = {}

            def _qt(c):
                if c not in qt_d:
                    qt_d[c] = qtp.tile([128, 4, 512], BF16D,
                                       name=f"qt{c}", tag="qt")
                return qt_d[c]

            def qpiece(c, dt, tag="mm"):
                """Q projection for chunk c, one d-tile (one [128,512] psum),
                reading the chunk-sliced xq tiles."""
                qt_c = _qt(c)
                pool = mmps if tag == "mm" else accp
                ps = pool.tile([128, 512], F32, name="qps", tag=tag)
                for db in range(8):
                    nc.tensor.matmul(
                        ps,
                        wq_sb[:, db, dt * 128:(dt + 1) * 128],
                        xqs_d[c][db][:, :],
                        start=(db == 0), stop=(db == 7),
                    )
                nc.vector.tensor_scalar_add(
                    qt_c[:, dt, :], ps, bq_sb[:, dt:dt + 1],
                )

            def qpiece_parts(c, dt):
                """Q projection piece for (c, dt): 2 part-closures (~853ns PE
                each) sharing one [128,512] acc-ring psum."""
                state = {}

                def part(dbh):
                    if "ps" not in state:
                        state["qt"] = _qt(c)
                        state["ps"] = accp.tile([128, 512], F32,
                                                name="qpsw", tag="acc")
                    ps = state["ps"]
                    for db in range(4 * dbh, 4 * dbh + 4):
                        nc.tensor.matmul(
                            ps,
                            wq_sb[:, db, dt * 128:(dt + 1) * 128],
                            xqs_d[c][db][:, :],
                            start=(db == 0), stop=(db == 7),
                        )
                    if dbh == 1:
                        nc.vector.tensor_scalar_add(
                            state["qt"][:, dt, :], ps, bq_sb[:, dt:dt + 1],
                        )
                return [lambda dbh=dbh: part(dbh) for dbh in range(2)]

            def vpiece(st, xts, tag="mm"):
                """V projection for one sk tile (one [128,512] psum)."""
                pool = mmps if tag == "mm" else accp
                ps = pool.tile([128, DG], F32, name="vps", tag=tag)
                for db in range(8):
                    nc.tensor.matmul(
                        ps, xts[db][:, st * 128:(st + 1) * 128], wq_sb[:, db, :],
                        start=(db == 0), stop=(db == 7),
                    )
                nc.vector.tensor_add(
                    v_sb[:, st, :, 0:DH],
                    ps[:, :].rearrange("p (g e) -> p g e", g=HG),
                    bqrep,
                )

            at_d = {}

            def _at(c):
                if c not in at_d:
                    at_d[c] = atp.tile([128, 4, 512], BF16D,
                                       name=f"at{c}", tag="at")
                return at_d[c]

            def opiece_pair(c, kk, tag="acc"):
                """Output projection rows [c*512+kk*128, +128): both 512-col
                halves as 2 part-closures SHARING one [128,512] psum (the
                second group's matmuls WAR-wait on the first's copy-out)."""
                row = c * 512 + kk * 128
                state = {}

                def part(dch):
                    if "ps" not in state:
                        pool = accp if tag == "acc" else mmps
                        state["ps"] = pool.tile([128, 512], F32,
                                                name="opsw", tag=tag)
                    ops = state["ps"]
                    for pr in range(4):
                        nc.tensor.matmul(
                            ops,
                            at_d[c][:, pr, kk * 128:(kk + 1) * 128],
                            wo_sb[:, pr, dch * 512:(dch + 1) * 512],
                            start=(pr == 0), stop=(pr == 3),
                        )
                    osb = osbp.tile([128, 512], F32, name="osb", tag="osb")
                    nc.vector.tensor_copy(osb, ops)
                    nc.sync.dma_start(
                        out=out[row:row + 128, dch * 512:(dch + 1) * 512],
                        in_=osb)
                return [lambda dch=dch: part(dch) for dch in range(2)]

            # ---- lead-in --------------------------------------------------
            # only what pr0 of the stream needs up front: K d-tile 0, V
            # sk-tiles 0-8 and Q(c0) d-tile 0.  Everything else (V 9-15,
            # Q(c0) d-tiles 1-3, K d-tiles 1-3) drains into chunk-0
            # slot/window beats so the exp stream starts ~26us in.
            nmt_d = {p: [None] * NG for p in range(16)}
            nmq = [0]

            def emit_nmt_group(p, g, q=None):
                # gpsimd/sync split: HWDGE DMAs hold the issuing SEQ during
                # their sem waits, so the scalar (ACT) queue must stay clear
                # of mask traffic or the exp stream stalls ~1.6us per group
                c, pr = divmod(p, 4)
                nmt = nmp.tile([128, TG, 2, 512], BF16D, name="nmt", tag="nmt")
                if q is None:
                    q = nc.gpsimd if nmq[0] % 2 == 0 else nc.sync
                    nmq[0] += 1
                q.dma_start(out=nmt, in_=nm[c, pr, g])
                nmt_d[p][g] = nmt

            # wq halves first (they gate every projection), then xk and xv
            # striped across all three queues; pr0 masks go to gpsimd after
            # the x loads — they have ~20us of slack
            _wqr = wq.rearrange("(n p) m -> p n m", p=128)
            nc.sync.dma_start(out=wq_sb[:, 0:4, :], in_=_wqr[:, 0:4, :])
            nc.scalar.dma_start(out=wq_sb[:, 4:8, :], in_=_wqr[:, 4:8, :])
            xk_ts = load_x(xkT, "xk", queues=(nc.gpsimd, nc.scalar, nc.sync))
            xv_ts = load_x(xvT, "xv", queues=(nc.gpsimd, nc.sync, nc.scalar))
            load_xqs(0)
            for g in range(NG):
                emit_nmt_group(0, g, q=nc.gpsimd)
            _bqap = bq[:].rearrange("(g e) -> g e", g=HG)
            nc.scalar.dma_start(out=bqrep, in_=bass.AP(
                tensor=_bqap.tensor, offset=_bqap.offset,
                ap=[[0, 128]] + [list(d) for d in _bqap.ap]))
            load_xqs(1)
            nc.scalar.dma_start(out=wo_sb, in_=wo.rearrange("(n p) m -> p n m", p=128))

            # K d-tile 0 with the db contraction outermost: each db-round
            # consumes its xk tile as it lands (4 quarter-psums in flight)
            kps0 = [mmps.tile([128, 512], F32, name="kps", tag="mm"),
                    accp.tile([128, 512], F32, name="kps", tag="acc"),
                    accp.tile([128, 512], F32, name="kps", tag="acc"),
                    mmps.tile([128, 512], F32, name="kps", tag="mm")]
            for db in range(8):
                for quarter in range(4):
                    nc.tensor.matmul(
                        kps0[quarter],
                        wq_sb[:, db, 0:128],
                        xk_ts[db][:, quarter * 512:(quarter + 1) * 512],
                        start=(db == 0), stop=(db == 7),
                    )
            for quarter in range(4):
                nc.vector.tensor_scalar_add(
                    kt_sb[:, 0, quarter * 512:(quarter + 1) * 512],
                    kps0[quarter], bq_sb[:, 0:1],
                )
            # V 0-6 in db-interleaved pairs; Q(c0) dt0 rides the V2 round so
            # its bias-add latency hides under the V4-V6 matmuls
            qt0 = _qt(0)
            q0ps = None
            vps = {}
            for st0 in (0, 2, 4):
                vps[st0] = [mmps.tile([128, DG], F32, name="vps", tag="mm"),
                            accp.tile([128, DG], F32, name="vps", tag="acc")]
                if st0 == 2:
                    q0ps = accp.tile([128, 512], F32, name="qps", tag="acc")
                for db in range(8):
                    for i in range(2):
                        nc.tensor.matmul(
                            vps[st0][i],
                            xv_ts[db][:, (st0 + i) * 128:(st0 + i + 1) * 128],
                            wq_sb[:, db, :],
                            start=(db == 0), stop=(db == 7),
                        )
                    if st0 == 2:
                        nc.tensor.matmul(
                            q0ps, wq_sb[:, db, 0:128], xqs_d[0][db][:, :],
                            start=(db == 0), stop=(db == 7),
                        )
                for i in range(2):
                    nc.vector.tensor_add(
                        v_sb[:, st0 + i, :, 0:DH],
                        vps[st0][i][:, :].rearrange("p (g e) -> p g e", g=HG),
                        bqrep,
                    )
                if st0 == 2:
                    nc.vector.tensor_scalar_add(qt0[:, 0, :], q0ps,
                                                bq_sb[:, 0:1])
            v6ps = mmps.tile([128, DG], F32, name="vps", tag="mm")
            for db in range(8):
                nc.tensor.matmul(
                    v6ps, xv_ts[db][:, 6 * 128:7 * 128], wq_sb[:, db, :],
                    start=(db == 0), stop=(db == 7),
                )
            nc.vector.tensor_add(
                v_sb[:, 6, :, 0:DH],
                v6ps[:, :].rearrange("p (g e) -> p g e", g=HG), bqrep)

            # ---- flattened beat stream -----------------------------------
            spair_d, probs_d, acc_d = {}, {}, {}

            def emit_s(b):
                c, pr, t = BEATS[b]
                sp = mmps.tile([128, 2, 512], F32, name="spair", tag="mm")
                spair_d[b] = sp
                tc_cols = slice(t * 128, (t + 1) * 128)
                nc.tensor.matmul(
                    sp[:, 0, :], kt_sb[0:64, pr, tc_cols],
                    qt_d[c][0:64, pr, :], start=True, stop=True,
                    tile_position=(0, 0),
                )
                nc.tensor.matmul(
                    sp[:, 1, :], kt_sb[64:128, pr, tc_cols],
                    qt_d[c][64:128, pr, :], start=True, stop=True,
                    tile_position=(64, 0),
                )

            def emit_exp_mask(b):
                c, pr, t = BEATS[b]
                p = 4 * c + pr
                probs = probsp.tile([128, 2, 512], BF16D, name="probs",
                                    tag="probs")
                probs_d[b] = probs
                nc.scalar.activation(probs, spair_d[b], EXP, scale=0.125)
                nc.vector.tensor_mul(probs, probs,
                                     nmt_d[p][t // TG][:, t % TG, :, :])

            def emit_av(b):
                c, pr, t = BEATS[b]
                p = 4 * c + pr
                if t == 0:
                    acc_d[p] = [accp.tile([128, 4, 128], F32, name="acc",
                                          tag="acc") for _ in range(2)]
                for h in range(2):
                    acc = acc_d[p][h]
                    for s in range(4):
                        nc.tensor.matmul(
                            acc[:, s, 0:DH + 1],
                            probs_d[b][:, h, s * 128:(s + 1) * 128],
                            v_sb[:, t, 2 * pr + h, :],
                            start=(t == 0 and s == 0),
                            stop=(t == NT - 1 and s == 3),
                        )

            def emit_norm(c, pr):
                p = 4 * c + pr
                at_c = _at(c)
                tmpt = tmpp.tile([128, 4, 2, DH], BF16D, name="tmpt",
                                 tag="tmpt")
                for h in range(2):
                    acc = acc_d[p][h]
                    rcp = rp.tile([128, 4], F32, name="rcp", tag="rcp")
                    nc.vector.reciprocal(rcp, acc[:, :, DH])
                    rap = rcp[:, :]
                    bc = bass.AP(tensor=rap.tensor, offset=rap.offset,
                                 ap=[list(rap.ap[0]), list(rap.ap[1]),
                                     [0, DH]])
                    nc.vector.tensor_mul(tmpt[:, :, h, :], acc[:, :, 0:DH], bc)
                # the last pr's transposes go to the scalar queue (idle once
                # the exp stream has ended; the sync queue is still draining
                # output stores, which would delay at(pr3) and the tail)
                q = nc.scalar if p == 15 else nc.sync
                for s in range(4):
                    q.dma_start(
                        out=at_c[:, pr, s * 128:(s + 1) * 128],
                        in_=tmpt[:, s, :, :], transpose=True)

            # chunk-0 slot pieces, ordered so every piece's EMISSION beat
            # precedes the emission of the first score matmul reading its
            # output (kt d-tile dt is read by scores of pr=dt, so K d-tiles
            # have progressive deadlines through chunk 0):
            #   pr0 t1..15 odd: Q(c0)dt1, V9..V15
            #   pr1 t1,3:      K dt1 cols 1536:2048, 1024:1536 (512-wide)
            #   pr1 t5..pr2 t3: K dt2 (256-wide x8)
            #   pr2 t5..pr3 t3: K dt3 (256-wide x8)
            small, win = [], []
            small.append(lambda: qpiece(0, 1))
            for st in range(7, NT):
                small.append(lambda st=st: vpiece(st, xv_ts, tag="mm"))
            small.append(lambda: kpiece(1, 2, xk_ts, width=512))
            small.append(lambda: kpiece(1, 3, xk_ts, width=512))
            for dt in (2, 3):
                for e in range(8):
                    small.append(lambda dt=dt, e=e:
                                 kpiece(dt, e, xk_ts, width=256))

            def drain1():
                if small:
                    small.pop(0)()

            next_s = 0
            for b in range(NB + LAG):
                if b < NB:
                    c, pr, t = BEATS[b]
                    p = 4 * c + pr
                    if t == 0 and pr == 0:
                        # queue this chunk's window pieces: 2-part pieces in
                        # acc-ring banks, allocated only at t in {5, 11}
                        # (ring order [h0, h1, w5, w11] so a window alloc
                        # never lands on a live accumulator)
                        if c == 0:
                            win.extend(kpair(1, 0, xk_ts))
                            win.extend(kpair(1, 2, xk_ts))
                            win.extend(qpiece_parts(0, 2))
                            win.extend(qpiece_parts(0, 3))
                            for dt in range(4):
                                win.extend(qpiece_parts(1, dt))
                        else:
                            if c + 1 < NC_:
                                for dt in range(4):
                                    win.extend(qpiece_parts(c + 1, dt))
                            for kk in range(4):
                                win.extend(opiece_pair(c - 1, kk))
                    # xq chunk-slice prefetch (two chunks ahead; keep these
                    # off the scalar queue so ACT SEQ never blocks)
                    if pr == 2 and t == 0 and c + 2 < NC_:
                        load_xqs(c + 2, queues=(nc.sync, nc.gpsimd))
                    # mask prefetch, one pr ahead, spread over even beats
                    if p + 1 < 16:
                        if t == 0:
                            emit_nmt_group(p + 1, 0)
                            emit_nmt_group(p + 1, 1)
                        elif t % 2 == 0 and t <= 12:
                            emit_nmt_group(p + 1, t // 2 + 1)
                    # mm drain slots (chunk 0 is PE-bound; ACT runs ahead)
                    slot_ok = t % 2 == 1 if c == 0 else t == 7
                    is_slot = slot_ok and small
                    # scores run 2 beats ahead of the exp stream (only to b
                    # on slot beats: the piece borrows the spair ring slot)
                    cap = b if is_slot else b + 2
                    while next_s <= cap and next_s < NB:
                        emit_s(next_s)
                        next_s += 1
                    emit_exp_mask(b)
                    if is_slot:
                        # one piece psum reuses a spair ring slot whose exp
                        # is in flight right now; scores catch up next beat
                        drain1()
                    if t in WIN_T and win:
                        win.pop(0)()
                if b >= LAG:
                    bb = b - LAG
                    emit_av(bb)
                    cb, prb, tb = BEATS[bb]
                    if tb == NT - 1:
                        emit_norm(cb, prb)
            while win:
                win.pop(0)()
            while small:
                small.pop(0)()
            # tail: output projection of the last chunk, 4 psum groups deep;
            # the pr3 contractions are emitted last so pr0-2 partials cover
            # the final norm/transpose latency.  ACT and Pool are idle once
            # the stream ends, so copies alternate DVE/ACT and stores
            # alternate sync/gpsimd to halve the drain.
            cl = NC_ - 1
            # 8 independent psum groups: kk2/kk3 share 2-bank mm units;
            # kk0/kk1 get four acc singles — the d0 pair evicts the spent
            # w5/w11 window banks, the d1 pair evicts h0/h1(pr3), whose WAR
            # (the final norm's reads) clears before these partials matter
            o_mm = [mmps.tile([128, 2, 512], F32, name="otp", tag="mm")
                    for _ in range(2)]
            o_ac = [accp.tile([128, 512], F32, name="otp", tag="acc")
                    for _ in range(4)]

            def _ops(kk, dch):
                if kk < 2:
                    return o_ac[kk + 2 * dch]
                return o_mm[kk - 2][:, dch, :]

            def _omm(kk, dch, pr, start, stop):
                nc.tensor.matmul(
                    _ops(kk, dch),
                    at_d[cl][:, pr, kk * 128:(kk + 1) * 128],
                    wo_sb[:, pr, dch * 512:(dch + 1) * 512],
                    start=start, stop=stop)

            def _oclose(kk, dch):
                _omm(kk, dch, 3, False, True)
                row = cl * 512 + kk * 128
                osb = osbp.tile([128, 512], F32, name="osbt", tag="osb")
                nc.vector.tensor_copy(osb, _ops(kk, dch))
                q = nc.sync if (kk + dch) % 2 == 0 else nc.gpsimd
                q.dma_start(
                    out=out[row:row + 128, dch * 512:(dch + 1) * 512],
                    in_=osb)

            # all pr0-2 partials run during the final beats; the 8 pr3
            # closers then chase at(pr3) with the copies pipelining behind
            for pr in range(3):
                for kk in range(4):
                    _omm(kk, 0, pr, pr == 0, False)
            for pr in range(3):
                for kk in range(4):
                    _omm(kk, 1, pr, pr == 0, False)
            for kk in range(4):
                _oclose(kk, 0)
            for kk in range(4):
                _oclose(kk, 1)

    nc.compile()
    return nc


_NC_CACHE = {}


def _get_nc(seq=SEQ):
    if seq not in _NC_CACHE:
        _NC_CACHE[seq] = build_nc(seq)
    return _NC_CACHE[seq]


def make_core_inputs(q, k, v, mask, W_q, b_q, W_o, seq=SEQ):
    """Build the 8 per-core input maps (host-side shard + layout)."""
    NT = seq // 128
    NC_ = seq // 512
    NG = NT // TG
    in_maps = []
    notm_all = (~np.asarray(mask)).astype(BF16)  # [B, 16, sq, sk]
    for core in range(8):
        b, gi = divmod(core, 2)
        cols = slice(gi * DG, (gi + 1) * DG)
        xqT = np.ascontiguousarray(np.asarray(q[b], np.float32).T.astype(BF16))
        xkT = np.ascontiguousarray(np.asarray(k[b], np.float32).T.astype(BF16))
        xvT = np.ascontiguousarray(np.asarray(v[b], np.float32).T.astype(BF16))
        wqc = np.ascontiguousarray(np.asarray(W_q, np.float32)[:, cols]).astype(BF16)
        bqc = np.ascontiguousarray(np.asarray(b_q, np.float32)[cols])
        woc = np.ascontiguousarray(np.asarray(W_o, np.float32)[cols, :]).astype(BF16)
        nmc = notm_all[b, gi * HG:(gi + 1) * HG]  # [8, sq, sk] bf16
        # [8h, sq, sk] -> [c, pr, g, p, t, h, j]
        # h -> (pr 4, h2); sq -> (c 4, j 512); sk -> (g NG, t TG, p 128)
        nmc = np.ascontiguousarray(
            nmc.reshape(4, 2, NC_, 512, NG, TG, 128)
               .transpose(2, 0, 4, 6, 5, 1, 3)
        )
        in_maps.append({
            "xqT": xqT, "xkT": xkT, "xvT": xvT,
            "wq": wqc, "bq": bqc, "wo": woc, "nm": nmc,
        })
    return in_maps


def kernel(q, k, v, mask, W_q, b_q, W_o, b_o):
    nc = _get_nc(SEQ)
    in_maps = make_core_inputs(q, k, v, mask, W_q, b_q, W_o, SEQ)
    res = run_bass_kernel_spmd(nc, in_maps, core_ids=list(range(8)))
    out = np.empty((BATCH, SEQ, D_MODEL), np.float32)
    bo = np.asarray(b_o, np.float32)
    for b in range(BATCH):
        out[b] = res.results[2 * b]["out"] + res.results[2 * b + 1]["out"] + bo
    return out
